# revision 41
# baseline (speedup 1.0000x reference)
"""Trainium2 Bass kernel for nn_AffinityMah (retrieval_knn).

Math (per batch b):
    out[n, m] = relu( ||Y[b,n] @ A||^2 + ||X[b,m] @ A||^2 - 2 * (YA @ XA^T)[n, m] )

Strategy:
  - Data-parallel over batch B=8 across the 8 NeuronCores (one batch per core).
  - Inputs are cast to bf16 AND pre-transposed on the host (X^T/Y^T with the
    contraction dim D on partitions) so the kernel needs no on-device
    transposes -- XA^T/YA^T slices come straight from matmuls against A
    chunks (contract D=256 in two 128-chunks, accumulated in PSUM).
  - The whole quadratic form is ONE TensorE matmul per (128, 512) output
    tile via an augmented contraction dim K+2 = 102 (YA^T/sqY/ones rows
    against -2XA^T/ones/sqX rows), giving sqY[:,None] + sqX[None,:] -
    2*cross directly in PSUM.  Squares run on the otherwise-idle GpSimd
    engine; row-sums via ones-vector matmuls; sq rows staged at partition 0
    and DMA'd into place (compute writes must be 32-aligned, DMAs need not).
  - Output is bf16 (host casts back to f32): halves output HBM traffic, the
    binding roofline for the back half.  Relu copies alternate ACT/DVE;
    output DMAs alternate sync HWDGE / gpsimd SWDGE queues.  Emission order
    interleaves stage-A units with output pairs (per-engine FIFOs follow
    emission order): front row-blocks drain as 256 KB half-row DMAs in
    wavefront order, the tail (j >= 8) as full 512 KB row DMAs with
    4 KB-contiguous lines, last row split across both queues.
  - A dummy ACT op at t~0 hoists the activation-table load off the
    critical path.
"""

import numpy as np

B, MX, NY, D, K = 8, 2048, 2048, 256, 100
KP = K + 2  # augmented contraction dim (sq/ones rows)
S = 512     # moving-operand slice width
NS = MX // S          # 4 column slices
JT = NY // 128        # 16 output row blocks

# relu tiles alternate ACT/DVE (both read f32 PSUM at ~1 elem/cycle/lane)
ACT_EVERY = 2

_NC = None


def _emit(tc, O, XT, YT, A):
    from contextlib import ExitStack

    import concourse.mybir as mybir

    nc = tc.nc
    f32 = mybir.dt.float32
    bf16 = mybir.dt.bfloat16
    AF = mybir.ActivationFunctionType
    ALU = mybir.AluOpType

    with ExitStack() as ctx:
        const = ctx.enter_context(tc.tile_pool(name="const", bufs=1))
        lr = ctx.enter_context(tc.tile_pool(name="lr", bufs=1))
        xin = ctx.enter_context(tc.tile_pool(name="xin", bufs=1))
        sqp = ctx.enter_context(tc.tile_pool(name="sqp", bufs=2))
        obp = ctx.enter_context(tc.tile_pool(name="obp", bufs=6))
        pa = ctx.enter_context(tc.tile_pool(name="pa", bufs=2, space="PSUM"))
        ps = ctx.enter_context(tc.tile_pool(name="ps", bufs=1, space="PSUM"))
        po = ctx.enter_context(tc.tile_pool(name="po", bufs=5, space="PSUM"))

        ones_w = const.tile([K, 1], bf16, name="ones_w", tag="ones_w")
        nc.vector.memset(ones_w[:], 1.0)
        ones_wx = const.tile([K, 1], bf16, name="ones_wx", tag="ones_wx")
        nc.vector.memset(ones_wx[:], 0.25)
        ones_row = const.tile([1, S], bf16, name="ones_row", tag="ones_row")
        nc.vector.memset(ones_row[:], 1.0)
        warm = const.tile([1, 1], bf16, name="warm", tag="warm")

        # hoist the ACT table load to t~0 (overlaps the input DMA)
        nc.scalar.activation(warm[:], ones_row[0:1, 0:1], AF.Relu)

        a_chunks = []
        for c in range(2):
            ac = const.tile([128, K], bf16, name=f"a{c}", tag=f"a{c}")
            nc.gpsimd.dma_start(ac[:], A[c * 128:(c + 1) * 128, :])
            a_chunks.append(ac)

        # L parts: [YA^T; sqY; ones], R parts: [-2 XA^T; ones; sqX]
        # constant ones rows staged once at t~0, off the critical path
        Lp, Rp = [], []
        for s in range(NS):
            lt = lr.tile([KP, S], bf16, name=f"L{s}", tag=f"L{s}")
            Lp.append(lt)
            rt = lr.tile([KP, S], bf16, name=f"R{s}", tag=f"R{s}")
            Rp.append(rt)
            nc.gpsimd.dma_start(lt[K + 1:K + 2, :], ones_row[:])
            nc.gpsimd.dma_start(rt[K:K + 1, :], ones_row[:])

        # ---- Input loads: host-pretransposed X^T/Y^T, D on partitions ----
        # Two 128-partition chunks per tensor, each loaded in two column
        # halves (256 KB DMAs) so stage A can start after ~512 KB.
        xts = {}   # (ti, c) -> [128, MX] tile; ti: 0=X, 1=Y
        for ti, T in ((0, XT), (1, YT)):
            for c in range(2):
                t_ = xin.tile([128, MX], bf16, name=f"in{ti}{c}", tag=f"in{ti}{c}")
                xts[ti, c] = t_
        H = MX // 2
        for h in range(2):
            for ti, T in ((0, XT), (1, YT)):
                for c in range(2):
                    nc.sync.dma_start(
                        xts[ti, c][:, h * H:(h + 1) * H],
                        T[c * 128:(c + 1) * 128, h * H:(h + 1) * H],
                    )

        # Engine wake-ups: the first op after an engine idles pays ~1.8 us of
        # semaphore wake latency; these tiny ops arm each engine's wait just
        # before its first real dependency arrives (ACT: the X input tile,
        # PE: the A chunks) so the real first op gets a warm back-to-back hop.
        wake = const.tile([1, 1], bf16, name="wake", tag="wake")
        nc.scalar.copy(wake[:], xts[0, 0][0:1, 0:1])
        pwake = po.tile([100, 100], f32, name="pwake", tag="po")
        nc.tensor.matmul(pwake[:], a_chunks[0][:], a_chunks[1][:],
                         start=True, stop=True)

        # ---- Stage A units + main-loop pairs, interleaved by dependency ----
        # Emission order fixes per-engine program order, so main-loop matmuls
        # must be emitted as soon as their L/R slices exist or the PE FIFO
        # serializes all of stage A ahead of them; conversely stage-A units
        # must be emitted ahead of the relu backlog that would starve them.
        relu_i = 0

        def emit_unit(ti, s):
            # XA^T / YA^T slice: accumulate over the two D-chunks
            pxa = pa.tile([K, S], f32, name=f"pxa{ti}{s}", tag="pa")
            nc.tensor.matmul(pxa[:], a_chunks[0][:],
                             xts[ti, 0][:, s * S:(s + 1) * S],
                             start=True, stop=False)
            nc.tensor.matmul(pxa[:], a_chunks[1][:],
                             xts[ti, 1][:, s * S:(s + 1) * S],
                             start=False, stop=True)

            # copy into L/R (ACT), square the bf16 copy (DVE 2x packed),
            # row-sum via a ones-matmul (LDWEIGHTS is 1 column ~ free),
            # stage the row at partition 0 and DMA it into place.
            if ti == 0:
                # (-2 XA)^2 * 0.25 = XA^2 via the 0.25-ones vector
                nc.scalar.mul(Rp[s][0:K, :], pxa[:], -2.0)
                sqt = sqp.tile([K, S], bf16, name=f"sq{ti}{s}", tag="sq")
                nc.gpsimd.tensor_mul(sqt[:], Rp[s][0:K, :], Rp[s][0:K, :])
                pss = ps.tile([1, S], f32, name=f"pss{ti}{s}", tag="ps")
                nc.tensor.matmul(pss[:], ones_wx[:], sqt[:], start=True, stop=True)
                sqrow = sqp.tile([1, S], bf16, name=f"sqrow{ti}{s}", tag="sqrow")
                nc.vector.tensor_copy(sqrow[:], pss[:])
                nc.sync.dma_start(Rp[s][K + 1:K + 2, :], sqrow[:])
            else:
                nc.scalar.copy(Lp[s][0:K, :], pxa[:])
                sqt = sqp.tile([K, S], bf16, name=f"sq{ti}{s}", tag="sq")
                nc.gpsimd.tensor_mul(sqt[:], Lp[s][0:K, :], Lp[s][0:K, :])
                pss = ps.tile([1, S], f32, name=f"pss{ti}{s}", tag="ps")
                nc.tensor.matmul(pss[:], ones_w[:], sqt[:], start=True, stop=True)
                sqrow = sqp.tile([1, S], bf16, name=f"sqrow{ti}{s}", tag="sqrow")
                nc.vector.tensor_copy(sqrow[:], pss[:])
                nc.sync.dma_start(Lp[s][K:K + 1, :], sqrow[:])

        def emit_tiles(j, ts, ot, ocol):
            # tiles t in ts for row-block j, relu'd into ot from column ocol
            nonlocal relu_i
            for i, t in enumerate(ts):
                on_act = relu_i % ACT_EVERY == 0
                pot = po.tile([128, S], f32, name=f"po{j}_{t}", tag="po")
                nc.tensor.matmul(
                    pot[:],
                    Lp[j // 4][:, (j % 4) * 128:(j % 4 + 1) * 128],
                    Rp[t][:],
                    start=True, stop=True,
                )
                c = (ocol + i) * S
                if on_act:
                    nc.scalar.activation(ot[:, c:c + S], pot[:], AF.Relu)
                else:
                    nc.vector.tensor_relu(ot[:, c:c + S], pot[:])
                relu_i += 1

        def emit_pair(j, th):
            # half-row: tiles {2th, 2th+1}, one 256 KB DMA
            ot = obp.tile([128, 2 * S], bf16, name=f"ot{j}_{th}", tag="ot")
            emit_tiles(j, [2 * th, 2 * th + 1], ot, 0)
            dma_eng = nc.sync if (j + th) % 2 == 0 else nc.gpsimd
            dma_eng.dma_start(
                O[j * 128:(j + 1) * 128, 2 * th * S:(2 * th + 2) * S], ot[:]
            )

        def emit_row(j):
            # full row: 4 tiles sharing one LDWEIGHTS, one 512 KB DMA with
            # 4 KB-contiguous per-partition lines
            ot = obp.tile([128, 4 * S], bf16, name=f"otr{j}", tag="otr")
            emit_tiles(j, [0, 1, 2, 3], ot, 0)
            dma_eng = nc.sync if j % 2 == 0 else nc.gpsimd
            dma_eng.dma_start(O[j * 128:(j + 1) * 128, :], ot[:])

        emit_unit(0, 0)           # R0
        emit_unit(0, 1)           # R1
        emit_unit(1, 0)           # L0
        for j in range(4):
            emit_pair(j, 0)
        emit_unit(1, 1)           # L1
        emit_unit(0, 2)           # R2
        emit_unit(0, 3)           # R3
        emit_unit(1, 2)           # L2
        for j in range(4, 8):
            emit_pair(j, 0)
        for j in range(4):
            emit_pair(j, 1)
        emit_unit(1, 3)           # L3
        for j in range(4, 8):
            emit_pair(j, 1)
        # tail: everything is ready, emit full rows (4 KB-contiguous 512 KB
        # DMAs, best descriptor efficiency); last row as two half-DMAs so the
        # final drain splits across both queues
        for j in range(8, 15):
            emit_row(j)
        emit_pair(15, 0)
        emit_pair(15, 1)


def _build_nc():
    import concourse.bass as bass  # noqa: F401
    import concourse.mybir as mybir
    import concourse.tile as tile
    from concourse import bacc

    bf16 = mybir.dt.bfloat16
    nc = bacc.Bacc(
        "TRN2", target_bir_lowering=False, debug=False, enable_asserts=False
    )
    XTd = nc.dram_tensor("XT", [D, MX], bf16, kind="ExternalInput").ap()
    YTd = nc.dram_tensor("YT", [D, NY], bf16, kind="ExternalInput").ap()
    Ad = nc.dram_tensor("A", [D, K], bf16, kind="ExternalInput").ap()
    Od = nc.dram_tensor("O", [NY, MX], bf16, kind="ExternalOutput").ap()

    with tile.TileContext(nc) as tc:
        _emit(tc, Od, XTd, YTd, Ad)
    nc.compile()
    return nc


def get_nc():
    global _NC
    if _NC is None:
        _NC = _build_nc()
    return _NC


def kernel(X, Y, A, _trace=False):
    import ml_dtypes

    from concourse.bass_utils import run_bass_kernel_spmd

    nc = get_nc()
    bf16 = ml_dtypes.bfloat16
    Xb = np.ascontiguousarray(X, dtype=np.float32).astype(bf16)
    Yb = np.ascontiguousarray(Y, dtype=np.float32).astype(bf16)
    Ab = np.ascontiguousarray(A, dtype=np.float32).astype(bf16)
    in_maps = [
        {
            "XT": np.ascontiguousarray(Xb[b].T),
            "YT": np.ascontiguousarray(Yb[b].T),
            "A": Ab,
        }
        for b in range(B)
    ]
    res = run_bass_kernel_spmd(nc, in_maps, core_ids=list(range(B)), trace=_trace)
    out = np.stack(
        [res.results[b]["O"].astype(np.float32) for b in range(B)], axis=0
    )
    if _trace:
        return out, res
    return out
